# revision 16
# baseline (speedup 1.0000x reference)
"""Bahdanau-attention GRU cell fused Trainium2 kernel (v2).

Sharding: data-parallel over batch across 8 NeuronCores (4 batch rows per
core, weights replicated, no collectives).

Staging (once per NEFF): annotations are DMA'd with 32KB-contiguous
per-partition descriptors (t = p*16 + w interleave), cast f32->bf16 by the
DGE, transposed on the PE to f-major a_T (bf16) and a_T8 (fp8) copies that
stay SBUF-resident. Weights load once and stay resident.

Per rep (b=4 local batch rows, T=2048, F=U=512):
  pre^T[u,t] = Ua^T ann^T (fp8 DoubleRow matmuls) ; tanh(+ Wa h + biases)
  fused on ScalarE with per-partition bias
  scores = Va . tanh(pre)  (PE, Va replicated across 128 partitions)
  p = exp(scores) (no max-sub; |scores| <= sum|Va| ~ 20, safe in fp32),
  Z via activation accum_out
  context^T[f] = sum_t a_T[f,t] p[t] via DVE tensor_tensor_reduce
  GRU gates: x@K + h@RK[:,:2U] + c@AK + biases in PSUM, hard-sigmoid/tanh
  epilogue, h_new = z*h + (1-z)*hh

The t index within a_T free dim is a fixed permutation of 0..T-1
(t = p*16 + w); softmax/context are permutation-invariant over t as long as
scores/p/context all use the same ordering, which they do by construction.
"""

import sys

if "/opt/trn_rl_repo" not in sys.path:
    sys.path.insert(0, "/opt/trn_rl_repo")

import numpy as np

import concourse.bass as bass
import concourse.tile as tile
from concourse import bacc, bass_utils, mybir
from concourse.masks import make_identity

F32 = mybir.dt.float32
BF16 = mybir.dt.bfloat16
FP8 = mybir.dt.float8e4
AF = mybir.ActivationFunctionType
ALU = mybir.AluOpType
DR = mybir.MatmulPerfMode.DoubleRow

B, T, F, U = 32, 2048, 512, 512
NCORES = 8
BL = B // NCORES          # 4 local batch rows
NFB = F // 128            # 4 f blocks
NUB = U // 128            # 4 u blocks
W = 16                    # t-rows per partition in the DMA layout
TH = 1024                 # T chunk for PSUM tiles (2 banks)
NTH = T // TH             # 2
U3 = 3 * U


def build(reps=1, pre_fp8=True, use_ttr=False):
    nc = bacc.Bacc("TRN2", target_bir_lowering=False, debug=False)

    def din(name, shape):
        return nc.dram_tensor(name, shape, F32, kind="ExternalInput").ap()

    d_x = din("x", [BL, F])
    d_h = din("h", [BL, U])
    d_ann = din("annotations", [BL, T, F])
    d_k = din("kernel", [F, U3])
    d_rk = din("recurrent_kernel", [U, U3])
    d_ak = din("attention_kernel", [F, U3])
    d_wa = din("Wa", [U, U])
    d_ua = din("Ua", [F, U])
    d_va = din("Va", [U])
    d_bias = din("bias", [U3])
    d_abias = din("attention_bias", [U3])
    d_wab = din("Wa_bias", [U])
    d_uab = din("Ua_bias", [U])
    d_out = nc.dram_tensor("h_new", [BL, U], F32, kind="ExternalOutput").ap()

    with tile.TileContext(nc) as tc:
        with (
            tc.tile_pool(name="const", bufs=1) as const,
            tc.tile_pool(name="annio", bufs=1) as annio,
            tc.tile_pool(name="tT_p", bufs=2) as tT_p,
            tc.tile_pool(name="pbc_p", bufs=2) as pbc_p,
            tc.tile_pool(name="scr_p", bufs=1) as scr_p,
            tc.tile_pool(name="scr_g", bufs=1) as scr_g,
            tc.tile_pool(name="qstate", bufs=2) as qstate,
            tc.tile_pool(name="state", bufs=1) as state,
            tc.tile_pool(name="ps_pp", bufs=2, space="PSUM") as ps_pp,
            tc.tile_pool(name="ps_sc", bufs=2, space="PSUM") as ps_sc,
        ):
            # ---------------- constants / weights ----------------
            ident = const.tile([128, 128], BF16)
            make_identity(nc, ident[:])
            ones4 = const.tile([1, BL], BF16)
            nc.vector.memset(ones4[:], 1.0)

            # annotations first: the big stream should start before weights
            ann_r = d_ann.rearrange("b (p w) f -> b p w f", p=128, w=W)
            HW = W // 4
            a_nat0 = annio.tile([128, HW, F], BF16, tag="nat", name="a_nat0")
            nc.gpsimd.dma_start(out=a_nat0[:], in_=ann_r[0, :, 0:HW, :])

            def row_load(dram_ap, width, nm):
                t16 = const.tile([1, width], BF16, name=nm)
                nc.gpsimd.dma_start(out=t16[:], in_=dram_ap)
                return t16

            va_row = row_load(d_va.rearrange("(a u) -> a u", a=1), U, "va_row")
            wab_row = row_load(d_wab.rearrange("(a u) -> a u", a=1), U, "wab_row")
            uab_row = row_load(d_uab.rearrange("(a u) -> a u", a=1), U, "uab_row")
            bias_row = row_load(d_bias.rearrange("(a u) -> a u", a=1), U3, "bias_row")
            abias_row = row_load(d_abias.rearrange("(a u) -> a u", a=1), U3,
                                 "abias_row")

            x_f32 = const.tile([BL, F], F32)
            nc.sync.dma_start(out=x_f32[:], in_=d_x)
            x_bf = const.tile([BL, F], BF16)
            nc.vector.tensor_copy(x_bf[:], x_f32[:])
            h_f32 = const.tile([BL, U], F32)
            nc.sync.dma_start(out=h_f32[:], in_=d_h)
            h_bf = const.tile([BL, U], BF16)
            nc.vector.tensor_copy(h_bf[:], h_f32[:])

            ua_sb = const.tile([128, NFB, U], BF16)
            nc.gpsimd.dma_start(
                out=ua_sb[:], in_=d_ua.rearrange("(fb p) u -> p fb u", p=128)
            )
            wa_sb = const.tile([128, NUB, U], BF16)
            nc.gpsimd.dma_start(
                out=wa_sb[:], in_=d_wa.rearrange("(jb p) u -> p jb u", p=128)
            )
            k_sb = const.tile([128, NFB, U3], BF16)
            nc.gpsimd.dma_start(
                out=k_sb[:], in_=d_k.rearrange("(fb p) u -> p fb u", p=128)
            )
            rk_sb = const.tile([128, NUB, U3], BF16)
            nc.gpsimd.dma_start(
                out=rk_sb[:], in_=d_rk.rearrange("(fb p) u -> p fb u", p=128)
            )
            ak_sb = const.tile([128, NFB, U3], BF16)
            nc.gpsimd.dma_start(
                out=ak_sb[:], in_=d_ak.rearrange("(fb p) u -> p fb u", p=128)
            )
            if pre_fp8:
                ua8 = const.tile([128, NFB, U], FP8)
                nc.vector.tensor_copy(ua8[:], ua_sb[:])

            # VaT replicated: va_rep[p, ub, j] = Va[ub*128+p] for all j
            va_rep = const.tile([128, NUB, 128], BF16)
            for ub in range(NUB):
                tp = ps_sc.tile([128, TH], BF16, tag="sc", name=f"vat{ub}")
                nc.tensor.transpose(
                    tp[:, 0:1], va_row[0:1, 128 * ub : 128 * (ub + 1)],
                    ident[0:1, 0:1],
                )
                nc.vector.tensor_copy(
                    va_rep[:, ub, :], tp[:, 0:1].to_broadcast([128, 128])
                )

            # x^T, h^T  (transpose [4,128] chunks -> [128,4])
            xT = const.tile([128, NFB, BL], BF16)
            hT = const.tile([128, NUB, BL], BF16)
            for jb in range(NFB):
                tp = ps_sc.tile([128, TH], BF16, tag="sc", name=f"xtt{jb}")
                nc.tensor.transpose(
                    tp[:, 0:BL], x_bf[0:BL, 128 * jb : 128 * (jb + 1)],
                    ident[0:BL, 0:BL],
                )
                nc.any.tensor_copy(xT[:, jb, :], tp[:, 0:BL])
            for jb in range(NUB):
                tp = ps_sc.tile([128, TH], BF16, tag="sc", name=f"htt{jb}")
                nc.tensor.transpose(
                    tp[:, 0:BL], h_bf[0:BL, 128 * jb : 128 * (jb + 1)],
                    ident[0:BL, 0:BL],
                )
                nc.any.tensor_copy(hT[:, jb, :], tp[:, 0:BL])

            # ---------------- annotation residency + transpose ----------
            a_T = const.tile([128, BL, NFB, T], BF16)
            if pre_fp8:
                a_T8 = const.tile([128, BL, NFB, T], FP8, name="a_T8")
            else:
                a_T8 = None
            for b in range(BL):
                for hf in range(4):
                    if b == 0 and hf == 0:
                        a_nat = a_nat0
                    else:
                        a_nat = annio.tile([128, HW, F], BF16, tag="nat",
                                           name=f"a_nat{b}_{hf}")
                        nc.gpsimd.dma_start(
                            out=a_nat[:],
                            in_=ann_r[b, :, hf * HW : (hf + 1) * HW, :],
                        )
                    for fb in range(NFB):
                        tp = ps_pp.tile([128, HW * 128], BF16, tag="pp",
                                        name=f"tp{b}_{hf}_{fb}")
                        for w in range(HW):
                            nc.tensor.transpose(
                                tp[:, 128 * w : 128 * (w + 1)],
                                a_nat[:, w, 128 * fb : 128 * (fb + 1)],
                                ident[:],
                            )
                        dst = slice(hf * HW * 128, (hf + 1) * HW * 128)
                        nc.vector.tensor_copy(a_T[:, b, fb, dst], tp[:])
                        if pre_fp8:
                            # GpSimd cannot read PSUM; cast from the SBUF copy
                            nc.gpsimd.tensor_copy(
                                a_T8[:, b, fb, dst], a_T[:, b, fb, dst]
                            )

            # ---------------- per-rep body ----------------
            for _rep in range(reps):
                # q^T[u, b] = Wa^T h^T + Wa_bias + Ua_bias
                qT = qstate.tile([128, NUB, BL], F32, tag="qT",
                                name=f"qT{_rep}")
                for ub in range(NUB):
                    qp = ps_sc.tile([128, TH], F32, tag="sc",
                                    name=f"qp{_rep}_{ub}")
                    for jb in range(NUB):
                        nc.tensor.matmul(
                            qp[:, 0:BL],
                            wa_sb[:, jb, 128 * ub : 128 * (ub + 1)],
                            hT[:, jb, :],
                            start=(jb == 0), stop=False,
                        )
                    nc.tensor.matmul(
                        qp[:, 0:BL],
                        wab_row[0:1, 128 * ub : 128 * (ub + 1)],
                        ones4[:], start=False, stop=False,
                    )
                    nc.tensor.matmul(
                        qp[:, 0:BL],
                        uab_row[0:1, 128 * ub : 128 * (ub + 1)],
                        ones4[:], start=False, stop=True,
                    )
                    nc.any.tensor_copy(qT[:, ub, :], qp[:, 0:BL])

                zt = qstate.tile([128, BL * NTH], F32, tag="zt",
                                name=f"zt{_rep}")
                cpart = qstate.tile([128, NFB, BL], F32, tag="cpart",
                                   name=f"cpart{_rep}")

                for b in range(BL):
                    pbc = pbc_p.tile([128, T], BF16, tag="pbc",
                                     name=f"pbc{_rep}_{b}")
                    for th in range(NTH):
                        tT = tT_p.tile([128, NUB, TH], BF16, tag="tT",
                                       name=f"tT{_rep}_{b}_{th}")
                        for ub in range(NUB):
                            pp = ps_pp.tile([128, TH], F32, tag="pp",
                                            name=f"pp{_rep}_{b}_{th}_{ub}")
                            for tq in range(TH // 512):
                                o = pp[:, 512 * tq : 512 * (tq + 1)]
                                tof = th * TH + tq * 512
                                if pre_fp8:
                                    for q in range(2):
                                        nc.tensor.matmul(
                                            o,
                                            ua8[:, 2 * q : 2 * q + 2,
                                                128 * ub : 128 * (ub + 1)],
                                            a_T8[:, b, 2 * q : 2 * q + 2,
                                                 tof : tof + 512],
                                            start=(q == 0), stop=(q == 1),
                                            perf_mode=DR,
                                        )
                                else:
                                    for fb in range(NFB):
                                        nc.tensor.matmul(
                                            o,
                                            ua_sb[:, fb,
                                                  128 * ub : 128 * (ub + 1)],
                                            a_T[:, b, fb, tof : tof + 512],
                                            start=(fb == 0),
                                            stop=(fb == NFB - 1),
                                        )
                            nc.scalar.activation(
                                tT[:, ub, :], pp[:], AF.Tanh,
                                bias=qT[:, ub, b : b + 1],
                            )
                        sc = ps_sc.tile([128, TH], F32, tag="sc",
                                        name=f"sc{_rep}_{b}_{th}")
                        for tq in range(TH // 512):
                            o = sc[:, 512 * tq : 512 * (tq + 1)]
                            for ub in range(NUB):
                                nc.tensor.matmul(
                                    o, va_rep[:, ub, :],
                                    tT[:, ub, 512 * tq : 512 * (tq + 1)],
                                    start=(ub == 0), stop=(ub == NUB - 1),
                                )
                        nc.scalar.activation(
                            pbc[:, th * TH : (th + 1) * TH], sc[:], AF.Exp,
                            accum_out=zt[:, b * NTH + th : b * NTH + th + 1],
                        )
                    # context partials via fused multiply+reduce on DVE
                    for fb in range(NFB):
                        pool = scr_p if fb < 2 else scr_g
                        scr = pool.tile([128, T], BF16, tag="scr",
                                        name=f"scr{_rep}_{b}_{fb}")
                        if use_ttr:
                            nc.vector.tensor_tensor_reduce(
                                out=scr[:],
                                in0=a_T[:, b, fb, :],
                                in1=pbc[:],
                                scale=1.0,
                                scalar=0.0,
                                op0=ALU.mult,
                                op1=ALU.add,
                                accum_out=cpart[:, fb, b : b + 1],
                            )
                        else:
                            # muls split DVE/GpSimd; GpSimd cannot reduce
                            # along the free axis, so DVE does all reduces
                            eng = nc.vector if fb < 2 else nc.gpsimd
                            eng.tensor_mul(scr[:], a_T[:, b, fb, :], pbc[:])
                            nc.vector.reduce_sum(
                                cpart[:, fb, b : b + 1], scr[:],
                                axis=mybir.AxisListType.X,
                            )

                # ---------------- softmax normalization ----------------
                zs = qstate.tile([128, BL], F32, tag="zs", name=f"zs{_rep}")
                for b in range(BL):
                    nc.vector.reduce_sum(
                        zs[:, b : b + 1],
                        zt[:, b * NTH : (b + 1) * NTH],
                        axis=mybir.AxisListType.X,
                    )
                rz = qstate.tile([128, BL], F32, tag="rz", name=f"rz{_rep}")
                nc.vector.reciprocal(rz[:], zs[:])
                cT = qstate.tile([128, NFB, BL], BF16, tag="cT",
                                name=f"cT{_rep}")
                for b in range(BL):
                    nc.vector.tensor_scalar(
                        out=cT[:, :, b],
                        in0=cpart[:, :, b],
                        scalar1=rz[:, b : b + 1],
                        scalar2=None,
                        op0=ALU.mult,
                    )

                # ---------------- GRU ----------------
                g_ps = []
                for nb in range(3):
                    pool = ps_pp if nb % 2 == 0 else ps_sc
                    tg = "pp" if nb % 2 == 0 else "sc"
                    gp = pool.tile([4, 512], F32, tag=tg,
                                   name=f"g_ps{_rep}_{nb}")
                    n0 = nb * 512
                    for fb in range(NFB):
                        nc.tensor.matmul(
                            gp[:], xT[:, fb, :], k_sb[:, fb, n0 : n0 + 512],
                            start=(fb == 0), stop=False,
                        )
                    if nb < 2:
                        for ub in range(NUB):
                            nc.tensor.matmul(
                                gp[:], hT[:, ub, :],
                                rk_sb[:, ub, n0 : n0 + 512],
                                start=False, stop=False,
                            )
                    nc.tensor.matmul(
                        gp[:], ones4[:], bias_row[0:1, n0 : n0 + 512],
                        start=False, stop=False,
                    )
                    nc.tensor.matmul(
                        gp[:], ones4[:], abias_row[0:1, n0 : n0 + 512],
                        start=False, stop=False,
                    )
                    for fb in range(NFB):
                        nc.tensor.matmul(
                            gp[:], cT[:, fb, :], ak_sb[:, fb, n0 : n0 + 512],
                            start=False, stop=(fb == NFB - 1),
                        )
                    g_ps.append(gp)

                def hard_sigmoid(dst, src, nm):
                    nc.vector.tensor_scalar(
                        out=dst, in0=src, scalar1=0.2, scalar2=0.5,
                        op0=ALU.mult, op1=ALU.add,
                    )
                    nc.vector.tensor_scalar(
                        out=dst, in0=dst, scalar1=0.0, scalar2=1.0,
                        op0=ALU.max, op1=ALU.min,
                    )

                z_sb = state.tile([BL, U], F32, name=f"z_sb{_rep}", tag="z_sb")
                r_sb = state.tile([BL, U], F32, name=f"r_sb{_rep}", tag="r_sb")
                hard_sigmoid(z_sb[:], g_ps[0][:], "z")
                hard_sigmoid(r_sb[:], g_ps[1][:], "r")

                rh_bf = state.tile([BL, U], BF16, name=f"rh_bf{_rep}",
                                   tag="rh_bf")
                nc.vector.tensor_mul(rh_bf[:], r_sb[:], h_f32[:])
                rhT = qstate.tile([128, NUB, BL], BF16, name=f"rhT{_rep}",
                                 tag="rhT")
                for ub in range(NUB):
                    tp = ps_pp.tile([128, T], BF16, tag="pp",
                                    name=f"tpg{_rep}_{ub}")
                    nc.tensor.transpose(
                        tp[:, 0:BL], rh_bf[0:BL, 128 * ub : 128 * (ub + 1)],
                        ident[0:BL, 0:BL],
                    )
                    nc.any.tensor_copy(rhT[:, ub, :], tp[:, 0:BL])

                hh_ps = ps_sc.tile([4, 512], F32, tag="sc",
                                   name=f"hh_ps{_rep}")
                for ub in range(NUB):
                    nc.tensor.matmul(
                        hh_ps[:], rhT[:, ub, :], rk_sb[:, ub, 2 * U : 3 * U],
                        start=(ub == 0), stop=(ub == NUB - 1),
                    )

                xh_sb = state.tile([BL, U], F32, name=f"xh_sb{_rep}",
                                   tag="xh_sb")
                nc.any.tensor_copy(xh_sb[:], g_ps[2][:])
                hh_pre = state.tile([BL, U], F32, name=f"hh_pre{_rep}",
                                    tag="hh_pre")
                nc.vector.tensor_add(hh_pre[:], xh_sb[:], hh_ps[:])
                hh = state.tile([BL, U], F32, name=f"hh{_rep}", tag="hh")
                nc.scalar.activation(hh[:], hh_pre[:], AF.Tanh)

                # h_new = hh + z * (h - hh)
                d_sb = state.tile([BL, U], F32, name=f"d_sb{_rep}", tag="xh_sb")
                nc.vector.tensor_sub(d_sb[:], h_f32[:], hh[:])
                zd = state.tile([BL, U], F32, name=f"zd{_rep}", tag="hh_pre")
                nc.vector.tensor_mul(zd[:], z_sb[:], d_sb[:])
                out_sb = state.tile([BL, U], F32, name=f"out_sb{_rep}",
                                    tag="z_sb")
                nc.vector.tensor_add(out_sb[:], hh[:], zd[:])
                nc.sync.dma_start(out=d_out, in_=out_sb[:])

    nc.compile()
    return nc


_NC = None


def _get_nc():
    global _NC
    if _NC is None:
        _NC = build()
    return _NC


def kernel(**inputs):
    nc = _get_nc()
    shared = {
        k: np.ascontiguousarray(np.asarray(inputs[k], np.float32))
        for k in (
            "kernel", "recurrent_kernel", "attention_kernel", "Wa", "Ua", "Va",
            "bias", "attention_bias", "Wa_bias", "Ua_bias",
        )
    }
    in_maps = []
    for c in range(NCORES):
        sl = slice(c * BL, (c + 1) * BL)
        m = dict(shared)
        m["x"] = np.ascontiguousarray(np.asarray(inputs["x"], np.float32)[sl])
        m["h"] = np.ascontiguousarray(np.asarray(inputs["h"], np.float32)[sl])
        m["annotations"] = np.ascontiguousarray(
            np.asarray(inputs["annotations"], np.float32)[sl]
        )
        in_maps.append(m)
    res = bass_utils.run_bass_kernel_spmd(nc, in_maps, core_ids=list(range(NCORES)))
    return np.concatenate([r["h_new"] for r in res.results], axis=0)


# revision 29
# speedup vs baseline: 2.3091x; 2.3091x over previous
"""Bahdanau-attention GRU cell fused Trainium2 kernel (v3).

Sharding: data-parallel over batch across 8 NeuronCores (4 batch rows per
core, weights replicated, no collectives).

Staging (once per NEFF): annotations are DMA'd with 8KB-contiguous
per-partition descriptors (t = p*16 + w interleave), cast f32->bf16 by the
DGE, transposed on the PE to f-major fp8 a_T8 that stays SBUF-resident.
Weights load once and stay resident (bf16 + fp8 copies).

Per rep (b=4 local batch rows, T=2048, F=U=512):
  pre^T[u,t] = Ua^T ann^T   (fp8 DoubleRow matmuls, T-wide free dim)
  tanh(pre + Wa h + biases) fused on ScalarE with per-partition bias,
  fp8 output
  scores = Va . tanh(pre)   (fp8 DoubleRow, Va replicated across partitions)
  p = exp(scores)  (no max-sub; |scores| <= sum|Va| ~ 20, safe in fp32),
  Z via activation accum_out
  context^T[f] = sum_t a_T8[f,t] p[t] via one fused DVE pass per f-block
  (scalar_tensor_tensor with accum_out); per-b normalization and immediate
  fp8 DoubleRow accumulation of c @ AK into the persistent z/r gate PSUM so
  the tensor engine never waits for the end of the batch loop
  GRU: x@K + h@RK[:,:2U] + biases accumulate at rep start; hard-sigmoid /
  tanh epilogue, h_new = z*h + (1-z)*hh

The t index within a_T8's free dim is a fixed permutation of 0..T-1
(t = p*16 + w); softmax/context are permutation-invariant over t as long as
scores/p/context all use the same ordering, which they do by construction.
"""

import sys

if "/opt/trn_rl_repo" not in sys.path:
    sys.path.insert(0, "/opt/trn_rl_repo")

import numpy as np

import concourse.bass as bass
import concourse.tile as tile
from concourse import bacc, bass_utils, mybir
from concourse.masks import make_identity

F32 = mybir.dt.float32
BF16 = mybir.dt.bfloat16
FP8 = mybir.dt.float8e4
AF = mybir.ActivationFunctionType
ALU = mybir.AluOpType
DR = mybir.MatmulPerfMode.DoubleRow

B, T, F, U = 32, 2048, 512, 512
NCORES = 8
BL = B // NCORES          # 4 local batch rows
NFB = F // 128            # 4 f blocks
NUB = U // 128            # 4 u blocks
W = 16                    # t-rows per partition in the DMA layout
TH = 1024                 # T chunk for PSUM tiles (2 banks)
NTH = T // TH             # 2
U3 = 3 * U


def build(reps=1, ctx_mul=False):
    """ctx_gp_fb: how many of the 4 context f-blocks run on GpSimd."""
    nc = bacc.Bacc("TRN2", target_bir_lowering=False, debug=False)

    def din(name, shape):
        return nc.dram_tensor(name, shape, F32, kind="ExternalInput").ap()

    d_x = din("x", [BL, F])
    d_h = din("h", [BL, U])
    d_ann = din("annotations", [BL, T, F])
    d_k = din("kernel", [F, U3])
    d_rk = din("recurrent_kernel", [U, U3])
    d_ak = din("attention_kernel", [F, U3])
    d_wa = din("Wa", [U, U])
    d_ua = din("Ua", [F, U])
    d_va = din("Va", [U])
    d_bias = din("bias", [U3])
    d_abias = din("attention_bias", [U3])
    d_wab = din("Wa_bias", [U])
    d_uab = din("Ua_bias", [U])
    d_out = nc.dram_tensor("h_new", [BL, U], F32, kind="ExternalOutput").ap()

    with tile.TileContext(nc) as tc:
        with (
            tc.tile_pool(name="const", bufs=1) as const,
            tc.tile_pool(name="annio", bufs=1) as annio,
            tc.tile_pool(name="tT_p", bufs=2) as tT_p,
            tc.tile_pool(name="pbc_p", bufs=2) as pbc_p,
            tc.tile_pool(name="scr_p", bufs=2) as scr_p,
            tc.tile_pool(name="qstate", bufs=2) as qstate,
            tc.tile_pool(name="state", bufs=2) as state,
            tc.tile_pool(name="ps_pp", bufs=3, space="PSUM") as ps_pp,
            tc.tile_pool(name="ps_g", bufs=1, space="PSUM") as ps_g,
        ):
            # ---------------- constants / weights ----------------
            ident = const.tile([128, 128], BF16)
            make_identity(nc, ident[:])
            ones4 = const.tile([1, BL], BF16)
            nc.vector.memset(ones4[:], 1.0)

            # annotations first: the big stream should start before weights
            ann_r = d_ann.rearrange("b (p w) f -> b p w f", p=128, w=W)
            HW = W // 4
            a_nat0 = annio.tile([128, HW, F], BF16, tag="nat", name="a_nat0")
            nc.gpsimd.dma_start(out=a_nat0[:], in_=ann_r[0, :, 0:HW, :])

            def row_load(dram_ap, width, nm):
                t16 = const.tile([1, width], BF16, name=nm)
                nc.gpsimd.dma_start(out=t16[:], in_=dram_ap)
                return t16

            va_row = row_load(d_va.rearrange("(a u) -> a u", a=1), U, "va_row")
            wab_row = row_load(d_wab.rearrange("(a u) -> a u", a=1), U, "wab_row")
            uab_row = row_load(d_uab.rearrange("(a u) -> a u", a=1), U, "uab_row")
            bias_row = row_load(d_bias.rearrange("(a u) -> a u", a=1), U3, "bias_row")
            abias_row = row_load(d_abias.rearrange("(a u) -> a u", a=1), U3,
                                 "abias_row")
            # combined rows: one bias matmul instead of two
            qbias_row = const.tile([1, U], BF16)
            nc.vector.tensor_add(qbias_row[:], wab_row[:], uab_row[:])
            gbias_row = const.tile([1, U3], BF16)
            nc.vector.tensor_add(gbias_row[:], bias_row[:], abias_row[:])

            x_f32 = const.tile([BL, F], F32)
            nc.sync.dma_start(out=x_f32[:], in_=d_x)
            x_bf = const.tile([BL, F], BF16)
            nc.vector.tensor_copy(x_bf[:], x_f32[:])
            h_f32 = const.tile([BL, U], F32)
            nc.sync.dma_start(out=h_f32[:], in_=d_h)
            h_bf = const.tile([BL, U], BF16)
            nc.vector.tensor_copy(h_bf[:], h_f32[:])

            ua_sb = const.tile([128, NFB, U], BF16)
            nc.gpsimd.dma_start(
                out=ua_sb[:], in_=d_ua.rearrange("(fb p) u -> p fb u", p=128)
            )
            wa_sb = const.tile([128, NUB, U], BF16)
            nc.gpsimd.dma_start(
                out=wa_sb[:], in_=d_wa.rearrange("(jb p) u -> p jb u", p=128)
            )
            k_sb = const.tile([128, NFB, U3], BF16)
            nc.gpsimd.dma_start(
                out=k_sb[:], in_=d_k.rearrange("(fb p) u -> p fb u", p=128)
            )
            rk_sb = const.tile([128, NUB, U3], BF16)
            nc.gpsimd.dma_start(
                out=rk_sb[:], in_=d_rk.rearrange("(fb p) u -> p fb u", p=128)
            )
            ak_sb = const.tile([128, NFB, U3], BF16)
            nc.gpsimd.dma_start(
                out=ak_sb[:], in_=d_ak.rearrange("(fb p) u -> p fb u", p=128)
            )
            ua8 = const.tile([128, NFB, U], FP8)
            nc.vector.tensor_copy(ua8[:], ua_sb[:])

            # VaT replicated (fp8): va_rep8[p, ub, j] = Va[ub*128+p] for all j
            va_rep8 = const.tile([128, NUB, 128], FP8)
            for ub in range(NUB):
                tp = ps_pp.tile([128, 512], BF16, tag="pp", name=f"vat{ub}")
                nc.tensor.transpose(
                    tp[:, 0:1], va_row[0:1, 128 * ub : 128 * (ub + 1)],
                    ident[0:1, 0:1],
                )
                nc.vector.tensor_copy(
                    va_rep8[:, ub, :], tp[:, 0:1].to_broadcast([128, 128])
                )

            # x^T, h^T  (transpose [4,128] chunks -> [128,4]) + fp8 copies
            xT = const.tile([128, NFB, BL], BF16)
            hT = const.tile([128, NUB, BL], BF16)
            for jb in range(NFB):
                tp = ps_pp.tile([128, 512], BF16, tag="pp", name=f"xtt{jb}")
                nc.tensor.transpose(
                    tp[:, 0:BL], x_bf[0:BL, 128 * jb : 128 * (jb + 1)],
                    ident[0:BL, 0:BL],
                )
                nc.vector.tensor_copy(xT[:, jb, :], tp[:, 0:BL])
            for jb in range(NUB):
                tp = ps_pp.tile([128, 512], BF16, tag="pp", name=f"htt{jb}")
                nc.tensor.transpose(
                    tp[:, 0:BL], h_bf[0:BL, 128 * jb : 128 * (jb + 1)],
                    ident[0:BL, 0:BL],
                )
                nc.vector.tensor_copy(hT[:, jb, :], tp[:, 0:BL])

            hT8 = const.tile([128, NUB, BL], FP8)
            nc.vector.tensor_copy(hT8[:], hT[:])
            wa8 = const.tile([128, NUB, U], FP8)
            nc.vector.tensor_copy(wa8[:], wa_sb[:])

            # ---------------- annotation residency + transpose ----------
            a_T = const.tile([128, BL, NFB, T], BF16)
            a_T8 = const.tile([128, BL, NFB, T], FP8)
            for b in range(BL):
                for hf in range(4):
                    if b == 0 and hf == 0:
                        a_nat = a_nat0
                    else:
                        a_nat = annio.tile([128, HW, F], BF16, tag="nat",
                                           name=f"a_nat{b}_{hf}")
                        nc.gpsimd.dma_start(
                            out=a_nat[:],
                            in_=ann_r[b, :, hf * HW : (hf + 1) * HW, :],
                        )
                    for fb in range(NFB):
                        tp = ps_pp.tile([128, 512], BF16, tag="pp",
                                        name=f"tp{b}_{hf}_{fb}")
                        for w in range(HW):
                            nc.tensor.transpose(
                                tp[:, 128 * w : 128 * (w + 1)],
                                a_nat[:, w, 128 * fb : 128 * (fb + 1)],
                                ident[:],
                            )
                        dst = slice(hf * HW * 128, (hf + 1) * HW * 128)
                        nc.vector.tensor_copy(
                            a_T[:, b, fb, dst], tp[:, 0 : HW * 128]
                        )
                        nc.gpsimd.tensor_copy(
                            a_T8[:, b, fb, dst], a_T[:, b, fb, dst]
                        )

            # ---------------- per-rep body ----------------
            def emit_qT(rep):
                """q^T[u, b] = Wa^T h^T + combined bias; qp tiles transient."""
                qT = qstate.tile([128, NUB, BL], F32, tag="qT",
                                 name=f"qT{rep}")
                for ub in range(NUB):
                    qp = ps_pp.tile([128, 512], F32, tag="pp",
                                    name=f"qp{rep}_{ub}")
                    for q in range(2):
                        nc.tensor.matmul(
                            qp[:, 0:BL],
                            wa8[:, 2 * q : 2 * q + 2,
                                128 * ub : 128 * (ub + 1)],
                            hT8[:, 2 * q : 2 * q + 2, :],
                            start=(q == 0), stop=False, perf_mode=DR,
                        )
                    nc.tensor.matmul(
                        qp[:, 0:BL],
                        qbias_row[0:1, 128 * ub : 128 * (ub + 1)],
                        ones4[:], start=False, stop=True,
                    )
                    nc.vector.tensor_copy(qT[:, ub, :], qp[:, 0:BL])
                return qT

            qT = emit_qT(0)
            deferred_close = None
            for _rep in range(reps):
                cT8 = qstate.tile([128, NFB, BL], FP8, tag="cT8",
                                  name=f"cT8{_rep}")
                cpart = qstate.tile([128, NFB, NTH, BL], F32, tag="cpart",
                                    name=f"cpart{_rep}")
                gzr = ps_g.tile([4, 2, 512], F32, tag="gzr",
                                name=f"gzr{_rep}")
                pbcs = {}
                ztbs = {}

                def emit_chunk_head(b, th, _rep=_rep):
                    """pre matmuls + tanh for chunk (b, th); returns tT8."""
                    tT8 = tT_p.tile([128, NUB, TH], FP8, tag="tT",
                                    name=f"tT{_rep}_{b}_{th}")
                    for ub in range(NUB):
                        pp = ps_pp.tile([128, TH], F32, tag="pp",
                                        name=f"pp{_rep}_{b}_{th}_{ub}")
                        for tq in range(TH // 512):
                            o = pp[:, 512 * tq : 512 * (tq + 1)]
                            tof = th * TH + tq * 512
                            for q in range(2):
                                nc.tensor.matmul(
                                    o,
                                    ua8[:, 2 * q : 2 * q + 2,
                                        128 * ub : 128 * (ub + 1)],
                                    a_T8[:, b, 2 * q : 2 * q + 2,
                                         tof : tof + 512],
                                    start=(q == 0), stop=(q == 1),
                                    perf_mode=DR,
                                )
                        nc.scalar.activation(
                            tT8[:, ub, :], pp[:], AF.Tanh,
                            bias=qT[:, ub, b : b + 1],
                        )
                    return tT8

                def emit_chunk_tail(b, th, tT8, _rep=_rep):
                    """scores + exp + context for chunk (b, th);
                    normalize + cT8 column after the second half."""
                    pbc = pbcs[b]
                    ztb = ztbs[b]
                    sc = ps_pp.tile([128, TH], F32, tag="pp",
                                    name=f"sc{_rep}_{b}_{th}")
                    for tq in range(TH // 512):
                        o = sc[:, 512 * tq : 512 * (tq + 1)]
                        for q in range(2):
                            nc.tensor.matmul(
                                o,
                                va_rep8[:, 2 * q : 2 * q + 2, :],
                                tT8[:, 2 * q : 2 * q + 2,
                                    512 * tq : 512 * (tq + 1)],
                                start=(q == 0), stop=(q == 1),
                                perf_mode=DR,
                            )
                    nc.scalar.activation(
                        pbc[:, th * TH : (th + 1) * TH], sc[:], AF.Exp,
                        accum_out=ztb[:, th : th + 1],
                    )
                    for fb in range(NFB):
                        scr = scr_p.tile([128, TH], BF16, tag="scr",
                                         name=f"scr{_rep}_{b}_{th}_{fb}")
                        if ctx_mul:
                            nc.vector.tensor_mul(
                                scr[:],
                                a_T[:, b, fb, th * TH : (th + 1) * TH],
                                pbc[:, th * TH : (th + 1) * TH],
                            )
                            nc.vector.tensor_scalar(
                                out=scr[:], in0=scr[:], scalar1=1.0,
                                scalar2=None, op0=ALU.mult,
                                accum_out=cpart[:, fb, th, b : b + 1],
                            )
                        else:
                            nc.vector.scalar_tensor_tensor(
                                out=scr[:],
                                in0=a_T[:, b, fb, th * TH : (th + 1) * TH],
                                scalar=1.0,
                                in1=pbc[:, th * TH : (th + 1) * TH],
                                op0=ALU.mult,
                                op1=ALU.mult,
                                accum_out=cpart[:, fb, th, b : b + 1],
                            )
                    if th == NTH - 1:
                        zsb = qstate.tile([128, 1], F32, tag="zsb",
                                          name=f"zsb{_rep}_{b}")
                        nc.vector.reduce_sum(zsb[:], ztb[:],
                                             axis=mybir.AxisListType.X)
                        rzb = qstate.tile([128, 1], F32, tag="rzb",
                                          name=f"rzb{_rep}_{b}")
                        nc.vector.reciprocal(rzb[:], zsb[:])
                        cs = qstate.tile([128, NFB], F32, tag="cs",
                                         name=f"cs{_rep}_{b}")
                        nc.vector.tensor_add(cs[:], cpart[:, :, 0, b],
                                             cpart[:, :, 1, b])
                        nc.vector.tensor_scalar(
                            out=cT8[:, :, b],
                            in0=cs[:],
                            scalar1=rzb[:],
                            scalar2=None,
                            op0=ALU.mult,
                        )

                def emit_gz_xh():
                    """x@K + h@RK + bias parts of z/r gates (no c dep)."""
                    for nb in range(2):
                        n0 = nb * 512
                        for fb in range(NFB):
                            nc.tensor.matmul(
                                gzr[:, nb, :], xT[:, fb, :],
                                k_sb[:, fb, n0 : n0 + 512],
                                start=(fb == 0), stop=False,
                                skip_group_check=True,
                            )
                        for ub in range(NUB):
                            nc.tensor.matmul(
                                gzr[:, nb, :], hT[:, ub, :],
                                rk_sb[:, ub, n0 : n0 + 512],
                                start=False, stop=False,
                                skip_group_check=True,
                            )
                        nc.tensor.matmul(
                            gzr[:, nb, :], ones4[:],
                            gbias_row[0:1, n0 : n0 + 512],
                            start=False, stop=False, skip_group_check=True,
                        )

                def make_tail_close(_rep=_rep, cT8=cT8, gzr=gzr):
                    def tail_close():
                        # fold c @ AK into the z/r gates
                        for nb in range(2):
                            n0 = nb * 512
                            for fb in range(NFB):
                                nc.tensor.matmul(
                                    gzr[:, nb, :], cT8[:, fb, :],
                                    ak_sb[:, fb, n0 : n0 + 512],
                                    start=False, stop=(fb == NFB - 1),
                                    skip_group_check=True,
                                )

                        def hard_sigmoid(dst, src):
                            nc.vector.tensor_scalar(
                                out=dst, in0=src, scalar1=0.2, scalar2=0.5,
                                op0=ALU.mult, op1=ALU.add,
                            )
                            nc.vector.tensor_scalar(
                                out=dst, in0=dst, scalar1=0.0, scalar2=1.0,
                                op0=ALU.max, op1=ALU.min,
                            )

                        z_sb = state.tile([BL, U], F32, name=f"z_sb{_rep}",
                                          tag="z_sb")
                        r_sb = state.tile([BL, U], F32, name=f"r_sb{_rep}",
                                          tag="r_sb")
                        hard_sigmoid(z_sb[:], gzr[:, 0, :])
                        hard_sigmoid(r_sb[:], gzr[:, 1, :])

                        rh_bf = state.tile([BL, U], BF16, name=f"rh_bf{_rep}",
                                           tag="rh_bf")
                        nc.vector.tensor_mul(rh_bf[:], r_sb[:], h_f32[:])
                        rhT = qstate.tile([128, NUB, BL], BF16,
                                          name=f"rhT{_rep}", tag="rhT")
                        for ub in range(NUB):
                            tp = ps_pp.tile([128, 512], BF16, tag="pp",
                                            name=f"tpg{_rep}_{ub}")
                            nc.tensor.transpose(
                                tp[:, 0:BL],
                                rh_bf[0:BL, 128 * ub : 128 * (ub + 1)],
                                ident[0:BL, 0:BL],
                            )
                            nc.vector.tensor_copy(rhT[:, ub, :], tp[:, 0:BL])

                        # hh gate: x@K3 + bias3 + c@AK3 + (r*h)@RK3
                        ghh = ps_pp.tile([4, 512], F32, tag="pp",
                                         name=f"ghh{_rep}")
                        n0 = 2 * 512
                        for fb in range(NFB):
                            nc.tensor.matmul(
                                ghh[:], xT[:, fb, :],
                                k_sb[:, fb, n0 : n0 + 512],
                                start=(fb == 0), stop=False,
                            )
                            nc.tensor.matmul(
                                ghh[:], cT8[:, fb, :],
                                ak_sb[:, fb, n0 : n0 + 512],
                                start=False, stop=False,
                            )
                            nc.tensor.matmul(
                                ghh[:], rhT[:, fb, :],
                                rk_sb[:, fb, n0 : n0 + 512],
                                start=False, stop=False,
                            )
                        nc.tensor.matmul(
                            ghh[:], ones4[:], gbias_row[0:1, n0 : n0 + 512],
                            start=False, stop=True,
                        )

                        hh = state.tile([BL, U], F32, name=f"hh{_rep}",
                                        tag="hh")
                        nc.scalar.activation(hh[:], ghh[:], AF.Tanh)

                        # h_new = hh + z * (h - hh)
                        d_sb = state.tile([BL, U], F32, name=f"d_sb{_rep}",
                                          tag="rh_bf")
                        nc.vector.tensor_sub(d_sb[:], h_f32[:], hh[:])
                        zd = state.tile([BL, U], F32, name=f"zd{_rep}",
                                        tag="rh_bf")
                        nc.vector.tensor_mul(zd[:], z_sb[:], d_sb[:])
                        out_sb = state.tile([BL, U], F32,
                                            name=f"out_sb{_rep}",
                                            tag="rh_bf")
                        nc.vector.tensor_add(out_sb[:], hh[:], zd[:])
                        nc.sync.dma_start(out=d_out, in_=out_sb[:])
                    return tail_close

                # software-pipelined chunk loop: scores/exp/context lag one
                # chunk behind pre/tanh; the previous rep's gate-close and
                # epilogue are deferred into this rep's chunk stream so the
                # PE never waits on the last context reduction
                chunks = [(b, th) for b in range(BL) for th in range(NTH)]
                pend = None
                for ci, (b, th) in enumerate(chunks):
                    if th == 0:
                        pbcs[b] = pbc_p.tile([128, T], BF16, tag="pbc",
                                             name=f"pbc{_rep}_{b}")
                        ztbs[b] = qstate.tile([128, NTH], F32, tag="ztb",
                                              name=f"ztb{_rep}_{b}")
                    tT8 = emit_chunk_head(b, th)
                    if ci == 4 and deferred_close is not None:
                        deferred_close()
                    if ci == len(chunks) - 2:
                        emit_gz_xh()
                    if pend is not None:
                        emit_chunk_tail(*pend)
                    pend = (b, th, tT8)
                emit_chunk_tail(*pend)

                # prefetch next rep's qT while the last context drains
                if _rep + 1 < reps:
                    qT = emit_qT(_rep + 1)
                deferred_close = make_tail_close()

            deferred_close()

    nc.compile()
    return nc


_NC = None


def _get_nc():
    global _NC
    if _NC is None:
        _NC = build()
    return _NC


def kernel(**inputs):
    nc = _get_nc()
    shared = {
        k: np.ascontiguousarray(np.asarray(inputs[k], np.float32))
        for k in (
            "kernel", "recurrent_kernel", "attention_kernel", "Wa", "Ua", "Va",
            "bias", "attention_bias", "Wa_bias", "Ua_bias",
        )
    }
    in_maps = []
    for c in range(NCORES):
        sl = slice(c * BL, (c + 1) * BL)
        m = dict(shared)
        m["x"] = np.ascontiguousarray(np.asarray(inputs["x"], np.float32)[sl])
        m["h"] = np.ascontiguousarray(np.asarray(inputs["h"], np.float32)[sl])
        m["annotations"] = np.ascontiguousarray(
            np.asarray(inputs["annotations"], np.float32)[sl]
        )
        in_maps.append(m)
    res = bass_utils.run_bass_kernel_spmd(nc, in_maps, core_ids=list(range(NCORES)))
    return np.concatenate([r["h_new"] for r in res.results], axis=0)


# revision 40
# speedup vs baseline: 2.4282x; 1.0516x over previous
"""Bahdanau-attention GRU cell fused Trainium2 kernel (v3).

Sharding: data-parallel over batch across 8 NeuronCores (4 batch rows per
core, weights replicated, no collectives).

Staging (once per NEFF): annotations are DMA'd with 8KB-contiguous
per-partition descriptors (t = p*16 + w interleave), cast f32->bf16 by the
DGE, transposed on the PE to f-major fp8 a_T8 that stays SBUF-resident.
Weights load once and stay resident (bf16 + fp8 copies).

Per rep (b=4 local batch rows, T=2048, F=U=512):
  pre^T[u,t] = Ua^T ann^T   (fp8 DoubleRow matmuls, T-wide free dim)
  tanh(pre + Wa h + biases) fused on ScalarE with per-partition bias,
  fp8 output
  scores = Va . tanh(pre)   (fp8 DoubleRow, Va replicated across partitions)
  p = exp(scores)  (no max-sub; |scores| <= sum|Va| ~ 20, safe in fp32),
  Z via activation accum_out
  context^T[f] = sum_t a_T8[f,t] p[t] via one fused DVE pass per f-block
  (scalar_tensor_tensor with accum_out); per-b normalization and immediate
  fp8 DoubleRow accumulation of c @ AK into the persistent z/r gate PSUM so
  the tensor engine never waits for the end of the batch loop
  GRU: x@K + h@RK[:,:2U] + biases accumulate at rep start; hard-sigmoid /
  tanh epilogue, h_new = z*h + (1-z)*hh

The t index within a_T8's free dim is a fixed permutation of 0..T-1
(t = p*16 + w); softmax/context are permutation-invariant over t as long as
scores/p/context all use the same ordering, which they do by construction.
"""

import sys

if "/opt/trn_rl_repo" not in sys.path:
    sys.path.insert(0, "/opt/trn_rl_repo")

import numpy as np

import concourse.bass as bass
import concourse.tile as tile
from concourse import bacc, bass_utils, mybir
from concourse.masks import make_identity

F32 = mybir.dt.float32
BF16 = mybir.dt.bfloat16
FP8 = mybir.dt.float8e4
AF = mybir.ActivationFunctionType
ALU = mybir.AluOpType
DR = mybir.MatmulPerfMode.DoubleRow

B, T, F, U = 32, 2048, 512, 512
NCORES = 8
BL = B // NCORES          # 4 local batch rows
NFB = F // 128            # 4 f blocks
NUB = U // 128            # 4 u blocks
W = 16                    # t-rows per partition in the DMA layout
TH = 1024                 # T chunk for PSUM tiles (2 banks)
NTH = T // TH             # 2
U3 = 3 * U


def build(reps=1, ctx_mul=False):
    """ctx_gp_fb: how many of the 4 context f-blocks run on GpSimd."""
    nc = bacc.Bacc("TRN2", target_bir_lowering=False, debug=False)

    def din(name, shape):
        return nc.dram_tensor(name, shape, F32, kind="ExternalInput").ap()

    d_x = din("x", [BL, F])
    d_h = din("h", [BL, U])
    d_ann = din("annotations", [BL, T, F])
    d_k = din("kernel", [F, U3])
    d_rk = din("recurrent_kernel", [U, U3])
    d_ak = din("attention_kernel", [F, U3])
    d_wa = din("Wa", [U, U])
    d_ua = din("Ua", [F, U])
    d_va = din("Va", [U])
    d_bias = din("bias", [U3])
    d_abias = din("attention_bias", [U3])
    d_wab = din("Wa_bias", [U])
    d_uab = din("Ua_bias", [U])
    d_out = nc.dram_tensor("h_new", [BL, U], F32, kind="ExternalOutput").ap()

    with tile.TileContext(nc) as tc:
        with (
            tc.tile_pool(name="const", bufs=1) as const,
            tc.tile_pool(name="annio", bufs=1) as annio,
            tc.tile_pool(name="tT_p", bufs=2) as tT_p,
            tc.tile_pool(name="pbc_p", bufs=2) as pbc_p,
            tc.tile_pool(name="scr_p", bufs=2) as scr_p,
            tc.tile_pool(name="qstate", bufs=2) as qstate,
            tc.tile_pool(name="state", bufs=2) as state,
            tc.tile_pool(name="ps_pp", bufs=4, space="PSUM") as ps_pp,
        ):
            # ---------------- constants / weights ----------------
            ident = const.tile([128, 128], BF16)
            make_identity(nc, ident[:])
            ones4 = const.tile([1, BL], BF16)
            nc.vector.memset(ones4[:], 1.0)

            # annotations first: the big stream should start before weights
            ann_r = d_ann.rearrange("b (p w) f -> b p w f", p=128, w=W)
            HW = W // 4
            a_nat0 = annio.tile([128, HW, F], BF16, tag="nat", name="a_nat0")
            nc.gpsimd.dma_start(out=a_nat0[:], in_=ann_r[0, :, 0:HW, :])

            def row_load(dram_ap, width, nm):
                t16 = const.tile([1, width], BF16, name=nm)
                nc.gpsimd.dma_start(out=t16[:], in_=dram_ap)
                return t16

            va_row = row_load(d_va.rearrange("(a u) -> a u", a=1), U, "va_row")
            wab_row = row_load(d_wab.rearrange("(a u) -> a u", a=1), U, "wab_row")
            uab_row = row_load(d_uab.rearrange("(a u) -> a u", a=1), U, "uab_row")
            bias_row = row_load(d_bias.rearrange("(a u) -> a u", a=1), U3, "bias_row")
            abias_row = row_load(d_abias.rearrange("(a u) -> a u", a=1), U3,
                                 "abias_row")
            # combined rows: one bias matmul instead of two
            qbias_row = const.tile([1, U], BF16)
            nc.vector.tensor_add(qbias_row[:], wab_row[:], uab_row[:])
            gbias_row = const.tile([1, U3], BF16)
            nc.vector.tensor_add(gbias_row[:], bias_row[:], abias_row[:])

            x_f32 = const.tile([BL, F], F32)
            nc.sync.dma_start(out=x_f32[:], in_=d_x)
            x_bf = const.tile([BL, F], BF16)
            nc.vector.tensor_copy(x_bf[:], x_f32[:])
            h_f32 = const.tile([BL, U], F32)
            nc.sync.dma_start(out=h_f32[:], in_=d_h)
            h_bf = const.tile([BL, U], BF16)
            nc.vector.tensor_copy(h_bf[:], h_f32[:])

            ua_sb = const.tile([128, NFB, U], BF16)
            nc.gpsimd.dma_start(
                out=ua_sb[:], in_=d_ua.rearrange("(fb p) u -> p fb u", p=128)
            )
            wa_sb = const.tile([128, NUB, U], BF16)
            nc.gpsimd.dma_start(
                out=wa_sb[:], in_=d_wa.rearrange("(jb p) u -> p jb u", p=128)
            )
            k_sb = const.tile([128, NFB, U3], BF16)
            nc.gpsimd.dma_start(
                out=k_sb[:], in_=d_k.rearrange("(fb p) u -> p fb u", p=128)
            )
            rk_sb = const.tile([128, NUB, U3], BF16)
            nc.gpsimd.dma_start(
                out=rk_sb[:], in_=d_rk.rearrange("(fb p) u -> p fb u", p=128)
            )
            ak_sb = const.tile([128, NFB, U3], BF16)
            nc.gpsimd.dma_start(
                out=ak_sb[:], in_=d_ak.rearrange("(fb p) u -> p fb u", p=128)
            )
            ua8 = const.tile([128, NFB, U], FP8)
            nc.vector.tensor_copy(ua8[:], ua_sb[:])

            # VaT replicated (fp8): va_rep8[p, ub, j] = Va[ub*128+p] for all j
            va_rep8 = const.tile([128, NUB, 128], FP8)
            for ub in range(NUB):
                tp = ps_pp.tile([128, 512], BF16, tag="pp", name=f"vat{ub}")
                nc.tensor.transpose(
                    tp[:, 0:1], va_row[0:1, 128 * ub : 128 * (ub + 1)],
                    ident[0:1, 0:1],
                )
                nc.vector.tensor_copy(
                    va_rep8[:, ub, :], tp[:, 0:1].to_broadcast([128, 128])
                )

            # x^T, h^T  (transpose [4,128] chunks -> [128,4]) + fp8 copies
            xT = const.tile([128, NFB, BL], BF16)
            hT = const.tile([128, NUB, BL], BF16)
            for jb in range(NFB):
                tp = ps_pp.tile([128, 512], BF16, tag="pp", name=f"xtt{jb}")
                nc.tensor.transpose(
                    tp[:, 0:BL], x_bf[0:BL, 128 * jb : 128 * (jb + 1)],
                    ident[0:BL, 0:BL],
                )
                nc.vector.tensor_copy(xT[:, jb, :], tp[:, 0:BL])
            for jb in range(NUB):
                tp = ps_pp.tile([128, 512], BF16, tag="pp", name=f"htt{jb}")
                nc.tensor.transpose(
                    tp[:, 0:BL], h_bf[0:BL, 128 * jb : 128 * (jb + 1)],
                    ident[0:BL, 0:BL],
                )
                nc.vector.tensor_copy(hT[:, jb, :], tp[:, 0:BL])

            hT8 = const.tile([128, NUB, BL], FP8)
            nc.vector.tensor_copy(hT8[:], hT[:])
            wa8 = const.tile([128, NUB, U], FP8)
            nc.vector.tensor_copy(wa8[:], wa_sb[:])

            # ---------------- annotation residency + transpose ----------
            a_T = const.tile([128, BL, NFB, T], BF16)
            a_T8 = const.tile([128, BL, NFB, T], FP8)
            for b in range(BL):
                for hf in range(4):
                    if b == 0 and hf == 0:
                        a_nat = a_nat0
                    else:
                        a_nat = annio.tile([128, HW, F], BF16, tag="nat",
                                           name=f"a_nat{b}_{hf}")
                        nc.gpsimd.dma_start(
                            out=a_nat[:],
                            in_=ann_r[b, :, hf * HW : (hf + 1) * HW, :],
                        )
                    for fb in range(NFB):
                        tp = ps_pp.tile([128, 512], BF16, tag="pp",
                                        name=f"tp{b}_{hf}_{fb}")
                        for w in range(HW):
                            nc.tensor.transpose(
                                tp[:, 128 * w : 128 * (w + 1)],
                                a_nat[:, w, 128 * fb : 128 * (fb + 1)],
                                ident[:],
                            )
                        dst = slice(hf * HW * 128, (hf + 1) * HW * 128)
                        nc.vector.tensor_copy(
                            a_T[:, b, fb, dst], tp[:, 0 : HW * 128]
                        )
                        nc.gpsimd.tensor_copy(
                            a_T8[:, b, fb, dst], a_T[:, b, fb, dst]
                        )

            # ---------------- per-rep body ----------------
            def emit_qT(rep):
                """q^T[u, b] = Wa^T h^T + combined bias; qp tiles transient."""
                qT = qstate.tile([128, NUB, BL], F32, tag="qT",
                                 name=f"qT{rep}")
                for ub in range(NUB):
                    qp = ps_pp.tile([128, 512], F32, tag="pp",
                                    name=f"qp{rep}_{ub}")
                    for q in range(2):
                        nc.tensor.matmul(
                            qp[:, 0:BL],
                            wa8[:, 2 * q : 2 * q + 2,
                                128 * ub : 128 * (ub + 1)],
                            hT8[:, 2 * q : 2 * q + 2, :],
                            start=(q == 0), stop=False, perf_mode=DR,
                        )
                    nc.tensor.matmul(
                        qp[:, 0:BL],
                        qbias_row[0:1, 128 * ub : 128 * (ub + 1)],
                        ones4[:], start=False, stop=True,
                    )
                    nc.vector.tensor_copy(qT[:, ub, :], qp[:, 0:BL])
                return qT

            qT = emit_qT(0)
            deferred_close = None
            for _rep in range(reps):
                cT8 = qstate.tile([128, NFB, BL], FP8, tag="cT8",
                                  name=f"cT8{_rep}")
                cpart = qstate.tile([128, NFB, NTH, BL], F32, tag="cpart",
                                    name=f"cpart{_rep}")
                pbcs = {}
                ztbs = {}

                def emit_chunk_head(b, th, _rep=_rep):
                    """pre matmuls + tanh for chunk (b, th); returns tT8."""
                    tT8 = tT_p.tile([128, NUB, TH], FP8, tag="tT",
                                    name=f"tT{_rep}_{b}_{th}")
                    for ub in range(NUB):
                        pp = ps_pp.tile([128, TH], F32, tag="pp",
                                        name=f"pp{_rep}_{b}_{th}_{ub}")
                        for tq in range(TH // 512):
                            o = pp[:, 512 * tq : 512 * (tq + 1)]
                            tof = th * TH + tq * 512
                            for q in range(2):
                                nc.tensor.matmul(
                                    o,
                                    ua8[:, 2 * q : 2 * q + 2,
                                        128 * ub : 128 * (ub + 1)],
                                    a_T8[:, b, 2 * q : 2 * q + 2,
                                         tof : tof + 512],
                                    start=(q == 0), stop=(q == 1),
                                    perf_mode=DR,
                                )
                        nc.scalar.activation(
                            tT8[:, ub, :], pp[:], AF.Tanh,
                            bias=qT[:, ub, b : b + 1],
                        )
                    return tT8

                def emit_chunk_tail(b, th, tT8, _rep=_rep):
                    """scores + exp + context for chunk (b, th);
                    normalize + cT8 column after the second half."""
                    pbc = pbcs[b]
                    ztb = ztbs[b]
                    sc = ps_pp.tile([128, TH], F32, tag="pp",
                                    name=f"sc{_rep}_{b}_{th}")
                    for tq in range(TH // 512):
                        o = sc[:, 512 * tq : 512 * (tq + 1)]
                        for q in range(2):
                            nc.tensor.matmul(
                                o,
                                va_rep8[:, 2 * q : 2 * q + 2, :],
                                tT8[:, 2 * q : 2 * q + 2,
                                    512 * tq : 512 * (tq + 1)],
                                start=(q == 0), stop=(q == 1),
                                perf_mode=DR,
                            )
                    nc.scalar.activation(
                        pbc[:, th * TH : (th + 1) * TH], sc[:], AF.Exp,
                        accum_out=ztb[:, th : th + 1],
                    )
                    for fb in range(NFB):
                        scr = scr_p.tile([128, TH], BF16, tag="scr",
                                         name=f"scr{_rep}_{b}_{th}_{fb}")
                        if ctx_mul:
                            nc.vector.tensor_mul(
                                scr[:],
                                a_T[:, b, fb, th * TH : (th + 1) * TH],
                                pbc[:, th * TH : (th + 1) * TH],
                            )
                            nc.vector.tensor_scalar(
                                out=scr[:], in0=scr[:], scalar1=1.0,
                                scalar2=None, op0=ALU.mult,
                                accum_out=cpart[:, fb, th, b : b + 1],
                            )
                        else:
                            nc.vector.scalar_tensor_tensor(
                                out=scr[:],
                                in0=a_T[:, b, fb, th * TH : (th + 1) * TH],
                                scalar=1.0,
                                in1=pbc[:, th * TH : (th + 1) * TH],
                                op0=ALU.mult,
                                op1=ALU.mult,
                                accum_out=cpart[:, fb, th, b : b + 1],
                            )
                    if th == NTH - 1:
                        zsb = qstate.tile([128, 1], F32, tag="zsb",
                                          name=f"zsb{_rep}_{b}")
                        nc.vector.reduce_sum(zsb[:], ztb[:],
                                             axis=mybir.AxisListType.X)
                        rzb = qstate.tile([128, 1], F32, tag="rzb",
                                          name=f"rzb{_rep}_{b}")
                        nc.vector.reciprocal(rzb[:], zsb[:])
                        cs = qstate.tile([128, NFB], F32, tag="cs",
                                         name=f"cs{_rep}_{b}")
                        nc.vector.tensor_add(cs[:], cpart[:, :, 0, b],
                                             cpart[:, :, 1, b])
                        nc.vector.tensor_scalar(
                            out=cT8[:, :, b],
                            in0=cs[:],
                            scalar1=rzb[:],
                            scalar2=None,
                            op0=ALU.mult,
                        )

                def make_tail_close(_rep=_rep, cT8=cT8):
                    def tail_close():
                        # z/r gates: complete accumulation groups in
                        # transient ring tiles (frees 2 PSUM banks for a
                        # 4th pre/score ring slot)
                        g_zr = []
                        for nb in range(2):
                            n0 = nb * 512
                            gp = ps_pp.tile([4, 512], F32, tag="pp",
                                            name=f"g{_rep}_{nb}")
                            for fb in range(NFB):
                                nc.tensor.matmul(
                                    gp[:], xT[:, fb, :],
                                    k_sb[:, fb, n0 : n0 + 512],
                                    start=(fb == 0), stop=False,
                                )
                            for ub in range(NUB):
                                nc.tensor.matmul(
                                    gp[:], hT[:, ub, :],
                                    rk_sb[:, ub, n0 : n0 + 512],
                                    start=False, stop=False,
                                )
                            nc.tensor.matmul(
                                gp[:], ones4[:],
                                gbias_row[0:1, n0 : n0 + 512],
                                start=False, stop=False,
                            )
                            for fb in range(NFB):
                                nc.tensor.matmul(
                                    gp[:], cT8[:, fb, :],
                                    ak_sb[:, fb, n0 : n0 + 512],
                                    start=False, stop=(fb == NFB - 1),
                                )
                            g_zr.append(gp)

                        def hard_sigmoid(dst, src):
                            nc.vector.tensor_scalar(
                                out=dst, in0=src, scalar1=0.2, scalar2=0.5,
                                op0=ALU.mult, op1=ALU.add,
                            )
                            nc.vector.tensor_scalar(
                                out=dst, in0=dst, scalar1=0.0, scalar2=1.0,
                                op0=ALU.max, op1=ALU.min,
                            )

                        z_sb = state.tile([BL, U], F32, name=f"z_sb{_rep}",
                                          tag="z_sb")
                        r_sb = state.tile([BL, U], F32, name=f"r_sb{_rep}",
                                          tag="r_sb")
                        hard_sigmoid(z_sb[:], g_zr[0][:])
                        hard_sigmoid(r_sb[:], g_zr[1][:])

                        rh_bf = state.tile([BL, U], BF16, name=f"rh_bf{_rep}",
                                           tag="rh_bf")
                        nc.vector.tensor_mul(rh_bf[:], r_sb[:], h_f32[:])
                        rhT = qstate.tile([128, NUB, BL], BF16,
                                          name=f"rhT{_rep}", tag="rhT")
                        for ub in range(NUB):
                            tp = ps_pp.tile([128, 512], BF16, tag="pp",
                                            name=f"tpg{_rep}_{ub}")
                            nc.tensor.transpose(
                                tp[:, 0:BL],
                                rh_bf[0:BL, 128 * ub : 128 * (ub + 1)],
                                ident[0:BL, 0:BL],
                            )
                            nc.vector.tensor_copy(rhT[:, ub, :], tp[:, 0:BL])

                        # hh gate: x@K3 + bias3 + c@AK3 + (r*h)@RK3
                        ghh = ps_pp.tile([4, 512], F32, tag="pp",
                                         name=f"ghh{_rep}")
                        n0 = 2 * 512
                        for fb in range(NFB):
                            nc.tensor.matmul(
                                ghh[:], xT[:, fb, :],
                                k_sb[:, fb, n0 : n0 + 512],
                                start=(fb == 0), stop=False,
                            )
                            nc.tensor.matmul(
                                ghh[:], cT8[:, fb, :],
                                ak_sb[:, fb, n0 : n0 + 512],
                                start=False, stop=False,
                            )
                            nc.tensor.matmul(
                                ghh[:], rhT[:, fb, :],
                                rk_sb[:, fb, n0 : n0 + 512],
                                start=False, stop=False,
                            )
                        nc.tensor.matmul(
                            ghh[:], ones4[:], gbias_row[0:1, n0 : n0 + 512],
                            start=False, stop=True,
                        )

                        hh = state.tile([BL, U], F32, name=f"hh{_rep}",
                                        tag="hh")
                        nc.scalar.activation(hh[:], ghh[:], AF.Tanh)

                        # h_new = hh + z * (h - hh)
                        d_sb = state.tile([BL, U], F32, name=f"d_sb{_rep}",
                                          tag="rh_bf")
                        nc.vector.tensor_sub(d_sb[:], h_f32[:], hh[:])
                        zd = state.tile([BL, U], F32, name=f"zd{_rep}",
                                        tag="rh_bf")
                        nc.vector.tensor_mul(zd[:], z_sb[:], d_sb[:])
                        out_sb = state.tile([BL, U], F32,
                                            name=f"out_sb{_rep}",
                                            tag="rh_bf")
                        nc.vector.tensor_add(out_sb[:], hh[:], zd[:])
                        nc.sync.dma_start(out=d_out, in_=out_sb[:])
                    return tail_close

                # software-pipelined chunk loop: scores/exp/context lag one
                # chunk behind pre/tanh; the previous rep's gate-close and
                # epilogue are deferred into this rep's chunk stream so the
                # PE never waits on the last context reduction
                chunks = [(b, th) for b in range(BL) for th in range(NTH)]
                pend = None
                for ci, (b, th) in enumerate(chunks):
                    if th == 0:
                        pbcs[b] = pbc_p.tile([128, T], BF16, tag="pbc",
                                             name=f"pbc{_rep}_{b}")
                        ztbs[b] = qstate.tile([128, NTH], F32, tag="ztb",
                                              name=f"ztb{_rep}_{b}")
                    tT8 = emit_chunk_head(b, th)
                    if ci == 4 and deferred_close is not None:
                        deferred_close()
                    if pend is not None:
                        emit_chunk_tail(*pend)
                    pend = (b, th, tT8)
                emit_chunk_tail(*pend)

                # prefetch next rep's qT while the last context drains
                if _rep + 1 < reps:
                    qT = emit_qT(_rep + 1)
                deferred_close = make_tail_close()

            deferred_close()

    nc.compile()
    return nc


_NC = None


def _get_nc():
    global _NC
    if _NC is None:
        _NC = build()
    return _NC


def kernel(**inputs):
    nc = _get_nc()
    shared = {
        k: np.ascontiguousarray(np.asarray(inputs[k], np.float32))
        for k in (
            "kernel", "recurrent_kernel", "attention_kernel", "Wa", "Ua", "Va",
            "bias", "attention_bias", "Wa_bias", "Ua_bias",
        )
    }
    in_maps = []
    for c in range(NCORES):
        sl = slice(c * BL, (c + 1) * BL)
        m = dict(shared)
        m["x"] = np.ascontiguousarray(np.asarray(inputs["x"], np.float32)[sl])
        m["h"] = np.ascontiguousarray(np.asarray(inputs["h"], np.float32)[sl])
        m["annotations"] = np.ascontiguousarray(
            np.asarray(inputs["annotations"], np.float32)[sl]
        )
        in_maps.append(m)
    res = bass_utils.run_bass_kernel_spmd(nc, in_maps, core_ids=list(range(NCORES)))
    return np.concatenate([r["h_new"] for r in res.results], axis=0)
